# revision 59
# baseline (speedup 1.0000x reference)
"""Trainium2 Bass kernel for nn_Attention_79207786873625.

Non-local attention block: 1x1 convs (theta/phi/g) -> maxpool2x2(phi,g) ->
scores = theta^T phi -> softmax over m -> o = g beta^T -> w_o conv ->
gamma*o + x.   Shapes: B=16, C=256, H=W=64 (n=HW=4096, m=HW/4=1024).

Sharding: data-parallel over batch across 8 cores (2 samples/core),
weights replicated, per-sample score matrix device-local.

Design (v2c):
- Scores sT[m, n] (m on partitions) in fp32r; exp in BF16 planar
  [128, 8 m-tiles, 1024 n] per n-quarter; attend gT(bf16) x exp(bf16)
  accumulates fp32 in PSUM at 1 cycle/row.
- Softmax denominator: instead of a second full PE pass (v1's
  ones-matmul over all 8 planes, 32k cycles/sample), the 8 exp planes
  are pair-summed on the DVE (bf16 tensor_tensor, 2x packed mode) down
  to 2 planes (P2), and a tiny 2-plane ones-matmul cross-partition-sums
  and broadcasts d.  PE denominator cost: 32768 -> 8192 cycles/sample.
- gamma is folded into the w_o weights at setup; the residual is a
  single tensor_tensor add (w_o PSUM result + x) per half-quarter.
- Critical-path schedule: the d-matmul is split around the iteration
  boundary (P2-half1 at iter end, half2 + reciprocal first thing next
  iter), w_o for sub1 and the out store are deferred past the boundary,
  sample-0 convs interleave with its first score tiles, and the g
  transposes are spread between attend accumulations.
- PSUM: tag "sT" [128,2,512] x2 bufs (scores, conv chunks, weight
  transposes) = 4 banks; tag "att" [128,512] x2 bufs (attend
  accumulators, g transposes) = 2 banks; tag "wo" [128,2,512] x1 buf
  (w_o outputs and the d accumulator, strictly rotated) = 2 banks.
"""
import sys

sys.path.insert(0, '/opt/trn_rl_repo')

from contextlib import ExitStack

import numpy as np

import concourse.bass as bass
import concourse.tile as tile
from concourse import bacc, mybir
from concourse.bass_utils import run_bass_kernel_spmd
from concourse.masks import make_identity

F32 = mybir.dt.float32
F32R = mybir.dt.float32r
BF16 = mybir.dt.bfloat16
AF = mybir.ActivationFunctionType
OP = mybir.AluOpType

B, C, H, W = 16, 256, 64, 64
HW = H * W            # 4096
M_POOL = HW // 4      # 1024
NCORES = 8
BPC = B // NCORES     # samples per core = 2


def build_kernel(nc, tc, ctx, x_d, wt_d, wp_d, wg_d, wo_d, gamma_d, out_d):
    sb = ctx.enter_context(tc.tile_pool(name="sb", bufs=1))
    per_s = ctx.enter_context(tc.tile_pool(name="per_s", bufs=2))
    stage1 = ctx.enter_context(tc.tile_pool(name="stage1", bufs=1))
    expp = ctx.enter_context(tc.tile_pool(name="expp", bufs=2))
    treep = ctx.enter_context(tc.tile_pool(name="treep", bufs=1))
    outp = ctx.enter_context(tc.tile_pool(name="outp", bufs=2))
    xp = ctx.enter_context(tc.tile_pool(name="xp", bufs=1))
    big = ctx.enter_context(tc.tile_pool(name="big", bufs=1, space="PSUM"))

    def load_x(b, eng, qs_list=(0, 1, 2, 3), halves=False):
        qs = []
        for qq in qs_list:
            x_t = xp.tile([128, 2, 1024], F32R, name="x_t", bufs=6)
            xv = x_d[b].rearrange("(c2 p) n -> p c2 n", p=128)
            if halves:
                for hx in range(2):
                    o = 1024 * qq + 512 * hx
                    eng.dma_start(
                        x_t[:, :, 512 * hx:512 * hx + 512],
                        xv[:, :, o:o + 512].bitcast(F32R),
                    )
            else:
                eng.dma_start(
                    x_t[:], xv[:, :, 1024 * qq:1024 * qq + 1024].bitcast(F32R),
                )
            qs.append(x_t)
        return qs

    # ---- constants (ordered to unblock convs ASAP) ----
    ident_f = sb.tile([128, 128], F32)
    make_identity(nc, ident_f[:])
    ident = sb.tile([128, 128], F32R)
    nc.vector.tensor_copy(ident[:], ident_f[:])

    # weights first: the first conv chunk needs wtp+wg before any x quarter
    wtp_nat = sb.tile([64, 256], F32R)
    nc.sync.dma_start(wtp_nat[0:32, :], wt_d.bitcast(F32R))
    nc.sync.dma_start(wtp_nat[32:64, :], wp_d.bitcast(F32R))
    wg_nat = sb.tile([128, 256], F32R)
    nc.sync.dma_start(wg_nat[:], wg_d.bitcast(F32R))
    wo_nat = sb.tile([128, 2, 128], F32R)
    nc.sync.dma_start(
        wo_nat[:], wo_d.rearrange("(two p) c -> p two c", p=128).bitcast(F32R)
    )
    gamma_bc = sb.tile([128, 1], F32)
    nc.sync.dma_start(gamma_bc[:], gamma_d.to_broadcast((128, 1)))
    x_q0 = xp.tile([128, 2, 1024], F32R, name="x_t", bufs=6)
    for _hx in range(2):
        nc.sync.dma_start(
            x_q0[:, :, 512 * _hx:512 * _hx + 512],
            x_d[0].rearrange("(c2 p) n -> p c2 n", p=128)[:, :, 512 * _hx:512 * _hx + 512].bitcast(F32R),
        )

    x_qs_next = [x_q0] + load_x(0, nc.sync, qs_list=(1, 2, 3), halves=True)

    wtp = sb.tile([128, 2, 64], F32R)     # [c_in_chunk, chunk, 64=theta|phi]
    wg = sb.tile([128, 2, 128], F32R)     # [c_in_chunk, chunk, 128 g-ch]
    for cc in range(2):
        trp_ps = big.tile([128, 2, 512], F32R, name="trp_ps", tag="sT", bufs=2)
        nc.tensor.transpose(
            trp_ps[:, 0, 0:64], wtp_nat[:, 128 * cc:128 * cc + 128], ident[0:64, 0:64]
        )
        nc.vector.tensor_copy(wtp[:, cc, :], trp_ps[:, 0, 0:64])
    for cc in range(2):
        trw_ps = big.tile([128, 2, 512], F32R, name="trw_ps", tag="sT", bufs=2)
        nc.tensor.transpose(
            trw_ps[:, 0, 0:128], wg_nat[:, 128 * cc:128 * cc + 128], ident[:]
        )
        nc.vector.tensor_copy(wg[:, cc, :], trw_ps[:, 0, 0:128])

    ones128 = sb.tile([128, 128], BF16)
    nc.vector.memset(ones128[:], 1.0)

    woT = sb.tile([128, 2, 128], F32R)    # [c(128), half, oc(128)], gamma-scaled
    wo_pending = [True]

    def emit_wo_transposes():
        if not wo_pending[0]:
            return
        wo_pending[0] = False
        for cc in range(2):
            trg_ps = big.tile([128, 2, 512], F32R, name="trg_ps", tag="wo", bufs=1)
            nc.tensor.transpose(trg_ps[:, 0, 0:128], wo_nat[:, cc, :], ident[:])
            # fold gamma into the w_o weights (out = (gamma*w_o) oU + x)
            nc.scalar.mul(
                woT[:, cc, :], trg_ps[:, 0, 0:128], gamma_bc[:, 0:1],
            )

    def make_state():
        st = {}
        st["theta_q"] = []
        phi = per_s.tile([32, M_POOL], F32R, name="phi")
        g_sb = per_s.tile([128, M_POOL], F32R, name="g_sb")
        gT = per_s.tile([128, 8, 128], BF16, name="gT", bufs=2)
        phi1 = stage1.tile([32, 64, 32], F32, name="phi1")
        g1 = stage1.tile([128, 64, 32], F32, name="g1")
        st["phi"], st["g_sb"], st["gT"], st["phi1"], st["g1"] = phi, g_sb, gT, phi1, g1
        return st

    def emit_gtr(st, t):
        # one 128-wide m-tile transpose of g (t in 0..7)
        g_sb, gT = st["g_sb"], st["gT"]
        gtr_ps = big.tile([128, 512], F32R, name="gtr_ps", tag="att", bufs=2)
        nc.tensor.transpose(
            gtr_ps[:, 0:128], g_sb[:, 128 * t:128 * t + 128], ident[:]
        )
        nc.vector.tensor_copy(gT[:, t, :], gtr_ps[:, 0:128].bitcast(F32))

    def emit_conv_chunk(st, x_qs_b, c4):
        phi, g_sb, phi1, g1 = st["phi"], st["g_sb"], st["phi1"], st["g1"]
        tp_ps = big.tile([64, 2, 512], F32, name="tp_ps", tag="sT", bufs=2)
        g_ps = big.tile([128, 2, 512], F32, name="g_ps", tag="sT", bufs=2)
        for sub in range(2):
            # per spatial half: theta/phi then g, so sub-0 compute overlaps
            # the second half's x DMA
            xoff = 512 * sub
            nc.tensor.matmul(
                tp_ps[:, sub, :], wtp[:, 0, :], x_qs_b[c4][:, 0, xoff:xoff + 512],
                start=True, stop=False,
            )
            nc.tensor.matmul(
                tp_ps[:, sub, :], wtp[:, 1, :], x_qs_b[c4][:, 1, xoff:xoff + 512],
                start=False, stop=True,
            )
            nc.tensor.matmul(
                g_ps[:, sub, :], wg[:, 0, :], x_qs_b[c4][:, 0, xoff:xoff + 512],
                start=True, stop=False,
            )
            nc.tensor.matmul(
                g_ps[:, sub, :], wg[:, 1, :], x_qs_b[c4][:, 1, xoff:xoff + 512],
                start=False, stop=True,
            )
        tpf = stage1.tile([64, 1024], F32R, name="tpf", bufs=4)
        nc.scalar.copy(tpf[:], tp_ps[:, :, :])
        st["theta_q"].append(tpf)
        pfv = tpf[32:64, :].bitcast(F32).rearrange("p (h w2 t) -> p h w2 t", w2=32, t=2)
        nc.vector.tensor_tensor(
            phi1[:, 16 * c4:16 * c4 + 16, :],
            pfv[:, :, :, 0], pfv[:, :, :, 1], op=OP.max,
        )
        gfv = g_ps.rearrange("p s (h w2 t) -> p (s h) w2 t", w2=32, t=2)
        g_odd = stage1.tile([128, 16, 32], F32, name="g_odd", bufs=2)
        nc.scalar.copy(g_odd[:], gfv[:, :, :, 1])
        nc.vector.tensor_tensor(
            g1[:, 16 * c4:16 * c4 + 16, :],
            gfv[:, :, :, 0], g_odd[:], op=OP.max,
        )
        # pool step 2 (rows) for this chunk
        p1v = phi1[:, 16 * c4:16 * c4 + 16, :].rearrange("p (i t) w -> p i t w", t=2)
        nc.vector.tensor_tensor(
            phi[:, 256 * c4:256 * c4 + 256].rearrange("p (i w) -> p i w", w=32),
            p1v[:, :, 0, :], p1v[:, :, 1, :], op=OP.max,
        )
        g1v = g1[:, 16 * c4:16 * c4 + 16, :].rearrange("p (i t) w -> p i t w", t=2)
        nc.vector.tensor_tensor(
            g_sb[:, 256 * c4:256 * c4 + 256].rearrange("p (i w) -> p i w", w=32),
            g1v[:, :, 0, :], g1v[:, :, 1, :], op=OP.max,
        )

    st = make_state()
    st_next = None
    pend_s1 = None       # (oUr, out_q, x_tile, b_idx, q_idx, sub)
    pend_s1_fine = [False]
    pend_gtr = []        # gtr tile indices deferred to next iter start

    def emit_pend_s1():
        nonlocal pend_s1
        if pend_s1 is None:
            return
        oUr_b, out_q_p, x_p, b_p, q_p, ssub = pend_s1
        fine = pend_s1_fine[0]
        pend_s1 = None
        soff = 512 * ssub
        wot1 = big.tile([128, 2, 512], F32, name="wot", tag="wo", bufs=1)
        for h in range(2):
            nc.tensor.matmul(
                wot1[:, h, :], woT[:, h, :], oUr_b[:],
                start=True, stop=True,
            )
        od = out_d[b_p].rearrange("(c2 p) n -> p c2 n", p=128)
        nqp = 1024 * q_p
        if fine:
            # drain: per-half residual+store so the last DMA overlaps DVE
            for h in range(2):
                nc.vector.tensor_tensor(
                    out_q_p[:, h, soff:soff + 512], wot1[:, h, :],
                    x_p[:, h, soff:soff + 512].bitcast(F32), op=OP.add,
                )
                nc.sync.dma_start(
                    od[:, h, nqp + soff:nqp + soff + 512],
                    out_q_p[:, h, soff:soff + 512]
                )
        else:
            nc.vector.tensor_tensor(
                out_q_p[:, :, soff:soff + 512], wot1[:, :, :],
                x_p[:, :, soff:soff + 512].bitcast(F32), op=OP.add,
            )
            nc.sync.dma_start(
                od[:, :, nqp + soff:nqp + soff + 512], out_q_p[:, :, soff:soff + 512]
            )

    expST_prev = None
    P2_prev = None
    P4_prev = None
    d_ps = None
    recip = None
    for b in range(BPC):
        x_qs = x_qs_next
        theta_q, phi, gT = st["theta_q"], st["phi"], st["gT"]
        emit_wo_transposes()
        # sample b>0's q0 scores were emitted inside the previous sample's
        # q4 iteration (merged slot); its pipeline state carries over.
        for q in (range(5) if b == 0 else range(1, 5)):
            # which quarter (and which sample's state) produces scores in
            # this iteration: q<4 -> own quarter q; q==4 -> next sample's q0
            if q < 4:
                sc_q, sc_st = q, st
            elif b + 1 < BPC:
                sc_q, sc_st = 0, st_next
            else:
                sc_q, sc_st = None, None
            expST = None
            P4 = P2 = None
            if sc_q is not None:
                expST = expp.tile([128, 8, 1024], BF16, name="expST")
                P4 = treep.tile([128, 4, 1024], BF16, name="P4", bufs=1)
                P2 = treep.tile([128, 1024], BF16, name="P2", bufs=1)
            out_q = None
            oUr_s1 = None
            att = {}
            if q == 1 and b + 1 < BPC:
                x_qs_next = load_x(b + 1, nc.scalar, qs_list=(0, 1))
            if q == 2 and b + 1 < BPC:
                x_qs_next = x_qs_next + load_x(b + 1, nc.scalar, qs_list=(2, 3))

            def emit_scores(u):
                sT_ps = big.tile([128, 2, 512], F32, name="sT_ps", tag="sT", bufs=2)
                for sub in range(2):
                    nc.tensor.matmul(
                        sT_ps[:, sub, :],
                        sc_st["phi"][:, 128 * u:128 * u + 128],
                        sc_st["theta_q"][sc_q][0:32, 512 * sub:512 * sub + 512],
                        start=True, stop=True,
                    )
                nc.scalar.activation(expST[:, u, :], sT_ps[:, :, :], AF.Exp)

            def emit_tree(u):
                # bf16 pair-sum tree toward the softmax denominator; emitted
                # late in each u-step so exp-dependent adds never block
                # ready DVE work (recip/oUr/out-tt) in the in-order queue.
                if u % 2 == 1:
                    nc.vector.tensor_tensor(
                        P4[:, u // 2, :], expST[:, u - 1, :], expST[:, u, :],
                        op=OP.add,
                    )
                if u == 3:
                    nc.vector.tensor_tensor(
                        P2[:], P4[:, 0, :], P4[:, 1, :], op=OP.add,
                    )

            for u in range(8):
                if sc_q is not None:
                    if b == 0 and q == 0 and u % 2 == 0:
                        # startup: interleave sample-0 convs with first scores
                        emit_conv_chunk(st, x_qs, u // 2)
                    emit_scores(u)
                if u == 1 and q >= 1:
                    # finish d(q-1): the last plane-pair (P4[3]) feeds straight
                    # from the final exp tiles, keeping the cross-iteration
                    # dependency chain short.
                    for dsub in range(2):
                        nc.tensor.matmul(
                            d_ps[:, dsub, :], ones128[:],
                            P4_prev[:, 3, 512 * dsub:512 * dsub + 512],
                            start=False, stop=True,
                        )
                    recip = outp.tile([128, 2, 512], F32, name="recip")
                    nc.vector.reciprocal_approx_fast(
                        out=recip[:, 0, :], in_=d_ps[:, 0, :]
                    )
                if u == 2 and q >= 1:
                    nc.vector.reciprocal_approx_fast(
                        out=recip[:, 1, :], in_=d_ps[:, 1, :]
                    )
                if u in (1, 2) and pend_gtr:
                    for t in pend_gtr[:2]:
                        emit_gtr(st, t)
                    del pend_gtr[:2]
                if u == 2:
                    emit_pend_s1()
                if q >= 1:
                    # on the very last iteration, process sub1 first so the
                    # drain tail only carries one sub's serial chain
                    swap = (b == BPC - 1 and q == 4)
                    sub = (1 - u // 4) if swap else (u // 4)
                    seg = u % 4  # seg: 2 m-tiles each
                    first = 1 if swap else 0
                    if seg == 0:
                        att[sub] = big.tile([128, 512], F32, name="att", tag="att", bufs=2)
                    for tl in range(2):
                        t = 2 * seg + tl
                        nc.tensor.matmul(
                            att[sub][:, :], gT[:, t, :],
                            expST_prev[:, t, 512 * sub:512 * sub + 512],
                            start=(t == 0), stop=(t == 7),
                        )
                    if u == 3:
                        oUr_a = outp.tile([128, 512], F32R, name="oUr")
                        nc.vector.scalar_tensor_tensor(
                            oUr_a[:], att[first][:, :], 1.0, recip[:, first, :],
                            op0=OP.mult, op1=OP.mult,
                        )
                    if u == 4:
                        wot0 = big.tile([128, 2, 512], F32, name="wot", tag="wo", bufs=1)
                        for h in range(2):
                            nc.tensor.matmul(
                                wot0[:, h, :], woT[:, h, :], oUr_a[:],
                                start=True, stop=True,
                            )
                    if u == 5:
                        out_q = outp.tile([128, 2, 1024], F32, name="out_q")
                        foff = 512 * first
                        nc.vector.tensor_tensor(
                            out_q[:, :, foff:foff + 512], wot0[:, :, :],
                            x_qs[q - 1][:, :, foff:foff + 512].bitcast(F32), op=OP.add,
                        )
                        od = out_d[b].rearrange("(c2 p) n -> p c2 n", p=128)
                        nqp = 1024 * (q - 1)
                        nc.sync.dma_start(
                            od[:, :, nqp + foff:nqp + foff + 512],
                            out_q[:, :, foff:foff + 512]
                        )
                    if u == 7:
                        second = 1 - first
                        oUr_b = outp.tile([128, 512], F32R, name="oUr")
                        nc.vector.scalar_tensor_tensor(
                            oUr_b[:], att[second][:, :], 1.0, recip[:, second, :],
                            op0=OP.mult, op1=OP.mult,
                        )
                        pend_s1 = (oUr_b, out_q, x_qs[q - 1], b, q - 1, second)
                # next sample's convs, spread one per iteration so no single
                # slot's PE/DVE (pools) load spikes
                if b + 1 < BPC and (q, u) in ((1, 5), (2, 5), (3, 5), (4, 1)):
                    if st_next is None:
                        st_next = make_state()
                    emit_conv_chunk(st_next, x_qs_next,
                                    {(1, 5): 0, (2, 5): 1, (3, 5): 2, (4, 1): 3}[(q, u)])
                if b == 0 and q == 0 and u in (3, 5):
                    k = (u - 3) // 2  # chunk 0 at u==3, chunk 1 at u==5
                    emit_gtr(st, 2 * k)
                    emit_gtr(st, 2 * k + 1)
                if b + 1 < BPC and (q, u) in ((2, 1), (3, 1), (4, 3)):
                    # next sample's g transposes, one chunk per iteration
                    k = q - 2
                    emit_gtr(st_next, 2 * k)
                    emit_gtr(st_next, 2 * k + 1)
                if sc_q is not None:
                    emit_tree(u)
                if sc_q is not None and u == 6:
                    # first accumulation legs of d(q)
                    d_ps = big.tile([128, 2, 512], F32, name="d_ps", tag="wo", bufs=1)
                    for dsub in range(2):
                        off = 512 * dsub
                        nc.tensor.matmul(
                            d_ps[:, dsub, :], ones128[:], P2[:, off:off + 512],
                            start=True, stop=False,
                        )
                        nc.tensor.matmul(
                            d_ps[:, dsub, :], ones128[:], P4[:, 2, off:off + 512],
                            start=False, stop=False,
                        )
            if b == 0 and q == 0:
                pend_gtr.extend([4, 5, 6, 7])
            if q == 4 and b + 1 < BPC:
                pend_gtr.extend([6, 7])
            expST_prev = expST
            P2_prev = P2
            P4_prev = P4
        if st_next is not None:
            st = st_next
            st_next = None
    pend_s1_fine[0] = True
    emit_pend_s1()


_CACHE = {}


def _get_compiled():
    if "nc" in _CACHE:
        return _CACHE["nc"]
    nc = bacc.Bacc("TRN2", target_bir_lowering=False, debug=False,
                   num_devices=NCORES)
    x_d = nc.dram_tensor("x", [BPC, C, HW], F32, kind="ExternalInput").ap()
    wt_d = nc.dram_tensor("w_theta", [32, 256], F32, kind="ExternalInput").ap()
    wp_d = nc.dram_tensor("w_phi", [32, 256], F32, kind="ExternalInput").ap()
    wg_d = nc.dram_tensor("w_g", [128, 256], F32, kind="ExternalInput").ap()
    wo_d = nc.dram_tensor("w_o", [256, 128], F32, kind="ExternalInput").ap()
    gamma_d = nc.dram_tensor("gamma", [1, 1], F32, kind="ExternalInput").ap()
    out_d = nc.dram_tensor("out", [BPC, C, HW], F32, kind="ExternalOutput").ap()

    with tile.TileContext(nc) as tc:
        with ExitStack() as ctx:
            build_kernel(nc, tc, ctx, x_d, wt_d, wp_d, wg_d, wo_d, gamma_d,
                         out_d)
    nc.compile()
    _CACHE["nc"] = nc
    return nc


def kernel(x, w_theta, w_phi, w_g, w_o, gamma, _trace=False, _tmpdir=None):
    nc = _get_compiled()
    x = np.ascontiguousarray(np.asarray(x, dtype=np.float32))
    in_maps = []
    for c in range(NCORES):
        shard = x[c * BPC:(c + 1) * BPC].reshape(BPC, C, HW)
        in_maps.append({
            "x": np.ascontiguousarray(shard),
            "w_theta": np.asarray(w_theta, np.float32),
            "w_phi": np.asarray(w_phi, np.float32),
            "w_g": np.asarray(w_g, np.float32),
            "w_o": np.asarray(w_o, np.float32),
            "gamma": np.asarray(gamma, np.float32).reshape(1, 1),
        })
    kwargs = {}
    if _trace:
        kwargs = dict(trace=True, tmpdir=_tmpdir)
    res = run_bass_kernel_spmd(nc, in_maps, core_ids=list(range(NCORES)),
                               **kwargs)
    out = np.concatenate([r["out"] for r in res.results], axis=0)
    out = out.reshape(B, C, H, W).astype(np.float32)
    if _trace:
        return out, res
    return out
